# revision 16
# baseline (speedup 1.0000x reference)
"""Trainium2 Bass kernel for flax MultiHeadDotProductAttention.

Shapes (hardcoded): B=4, Q=K=1500, D=1024, H=16, HD=64.
Sharding: 8 cores = 4 batches x 2 head-groups (8 heads each).
Each core computes its batch's attention output for its 8 heads plus the
output projection restricted to those heads; the host sums the two
head-group partials per batch and adds bo.

Dataflow per core (all layouts chosen so no on-device transposes are
needed; host passes x pre-transposed):
  qT/kT [hhd, seq] and v [seq, hhd] via projection matmuls;
  S^T[k,q] = kT.T-slices @ qT (K=64, row-packed 2 heads per PE slot);
  P^T = exp(S^T/8) on ScalarE (psum->sbuf, bf16);
  attn_outT += v_tile.T @ P^T (bf16, col-packed 2 heads per slot) and
  denominators via ones-vector matmuls (4 heads col-packed per slot),
  two head-pairs interleaved per k step so PE has independent work
  while ScalarE exponentiates; normalization via a selector matmul
  broadcast + one full-width approximate reciprocal; out-projection
  consumes the normalized [hhd, q] tiles as stationary operands ->
  natural [q, d] output tiles DMA'd straight to HBM.

MODE: "bf16" (default) runs all big matmuls in bf16 (weight loads
overlap in-flight matmuls); "mixed" keeps projections/S^T/out-proj in
fp32r (higher precision, but each matmul pays a serialized weight load).
"""

import os
import sys

sys.path.insert(0, "/opt/trn_rl_repo")

import numpy as np  # noqa: E402
import ml_dtypes  # noqa: E402
import concourse.bacc as bacc  # noqa: E402
import concourse.mybir as mybir  # noqa: E402
import concourse.tile as tile  # noqa: E402
from concourse.bass_utils import run_bass_kernel_spmd  # noqa: E402

F32 = mybir.dt.float32
F32R = mybir.dt.float32r
BF16 = mybir.dt.bfloat16
AF = mybir.ActivationFunctionType

B, SEQ, D, H, HD = 4, 1500, 1024, 16, 64
HG = 8                      # heads per group
HHD = HG * HD               # 512
DCH = D // 128              # 8 d-chunks
HB = HHD // 128             # 4 hhd blocks (2 heads each)
NPAIR = HB                  # 4 head pairs per group
QC = [(0, 512), (512, 512), (1024, 476)]          # q chunks
KT = [(i * 128, min(128, SEQ - i * 128)) for i in range((SEQ + 127) // 128)]
NKT = len(KT)               # 12 (last tile 92 rows)

MODE = os.environ.get("BASS_MM_DTYPE", "bf16")


def _build(mode, with_bias):
    mt = BF16 if mode == "bf16" else F32R          # big-matmul operand dtype
    MTD = BF16 if mode == "bf16" else F32          # dram dtype for x/w/b

    nc = bacc.Bacc("TRN2", target_bir_lowering=False, debug=False, num_devices=8)

    xqT = nc.declare_dram_parameter("xqT", [D, SEQ], MTD, isOutput=False)
    xkvT = nc.declare_dram_parameter("xkvT", [D, SEQ], MTD, isOutput=False)
    wq_d = nc.declare_dram_parameter("wq", [D, HHD], MTD, isOutput=False)
    wk_d = nc.declare_dram_parameter("wk", [D, HHD], MTD, isOutput=False)
    wv_d = nc.declare_dram_parameter("wv", [D, HHD], MTD, isOutput=False)
    wo_d = nc.declare_dram_parameter("wo", [HHD, D], MTD, isOutput=False)
    bq_d = nc.declare_dram_parameter("bq", [1, HHD], MTD, isOutput=False)
    bk_d = nc.declare_dram_parameter("bk", [1, HHD], MTD, isOutput=False)
    bv_d = nc.declare_dram_parameter("bv", [1, HHD], MTD, isOutput=False)
    sel_d = nc.declare_dram_parameter("sel", [128, 256], F32, isOutput=False)
    ones_d = nc.declare_dram_parameter("ones1", [1, 512], F32, isOutput=False)
    zr_d = nc.declare_dram_parameter("zr", [128, 512], F32, isOutput=False)
    out_d = nc.declare_dram_parameter("out", [SEQ, D], F32, isOutput=True)

    def mcast(ap):
        # view a dram param as the matmul dtype
        return ap if mode == "bf16" else ap.bitcast(F32R)

    with tile.TileContext(nc) as tc:
        from contextlib import ExitStack

        with ExitStack() as ctx:
            ctx.enter_context(nc.allow_low_precision(
                reason="bf16/f32r matmul operands; psum accumulation is fp32"
            ))
            const = ctx.enter_context(tc.tile_pool(name="const", bufs=1))
            ones_r = const.tile([1, 512], mt, tag="ones")
            if mode == "bf16":
                nc.vector.memset(ones_r[:], 1.0)
            else:
                nc.sync.dma_start(ones_r[:], ones_d[:].bitcast(F32R))
            dones = const.tile([128, 1], BF16, tag="dones")
            nc.vector.memset(dones[:], 1.0)
            sel_sb = const.tile([128, 256], F32R, tag="sel")
            nc.sync.dma_start(sel_sb[:], sel_d[:].bitcast(F32R))
            ds = const.tile([128, 512], F32R, tag="ds")
            nc.sync.dma_start(ds[:], zr_d[:].bitcast(F32R))
            bq_sb = const.tile([1, HHD], mt, tag="bq")
            nc.sync.dma_start(bq_sb[:], mcast(bq_d[:]))
            bk_sb = const.tile([1, HHD], mt, tag="bk")
            nc.sync.dma_start(bk_sb[:], mcast(bk_d[:]))
            bv_sb = const.tile([1, HHD], mt, tag="bv")
            nc.sync.dma_start(bv_sb[:], mcast(bv_d[:]))

            # persistent activations for the attention phase
            qT = [const.tile([128, SEQ], mt, tag="qT", name=f"qT{i}", bufs=HB)
                  for i in range(HB)]                       # [hhd%128, q] per blk
            kT = [const.tile([128, SEQ], mt, tag="kT", name=f"kT{i}", bufs=HB)
                  for i in range(HB)]
            v_sb = const.tile([128, NKT, HHD], BF16, tag="v")  # [k%128, ktile, hhd]

            wpool = ctx.enter_context(tc.tile_pool(name="w", bufs=2))

            def load_w(dram, cols):
                t = wpool.tile([128, D // 128, cols], mt, tag="w", bufs=2)
                nc.sync.dma_start(
                    t[:], mcast(dram.rearrange("(c p) n -> p c n", p=128))
                )
                return t

            # ---------------- phase 1: projections ----------------
            with tc.tile_pool(name="x", bufs=8) as xpool, \
                 tc.tile_pool(name="mmps", bufs=2, space="PSUM") as mm_ps:

                def load_x(dram):
                    xs = []
                    for c in range(DCH):
                        t = xpool.tile([128, SEQ], mt, tag="xc", bufs=8)
                        nc.sync.dma_start(
                            t[:], mcast(dram[c * 128:(c + 1) * 128, :])
                        )
                        xs.append(t)
                    return xs

                def proj_T(dst, w_sb, b_sb, xs):
                    # dst[:, hb, q] = (x @ W + b)^T rows hb*128..+128
                    for hb in range(HB):
                        for (qo, cw) in QC:
                            ps = mm_ps.tile([128, 512], F32, tag="mm", bufs=2)
                            for c in range(DCH):
                                nc.tensor.matmul(
                                    ps[:, :cw],
                                    w_sb[:, c, hb * 128:(hb + 1) * 128],
                                    xs[c][:, qo:qo + cw],
                                    start=(c == 0), stop=(not with_bias and c == DCH - 1),
                                )
                            if with_bias:
                                nc.tensor.matmul(
                                    ps[:, :cw],
                                    b_sb[0:1, hb * 128:(hb + 1) * 128],
                                    ones_r[0:1, :cw],
                                    start=False, stop=True,
                                )
                            nc.vector.tensor_copy(
                                dst[hb][:, qo:qo + cw], ps[:, :cw]
                            )

                def proj_v(dst, w_sb, b_sb, xs):
                    # dst[:, kt, hhd] = (x @ W + b) rows kt*128..
                    for kt, (ko, kh) in enumerate(KT):
                        ps = mm_ps.tile([128, 512], F32, tag="mm", bufs=2)
                        for c in range(DCH):
                            nc.tensor.matmul(
                                ps[:kh, :],
                                xs[c][:, ko:ko + kh],
                                w_sb[:, c, :],
                                start=(c == 0), stop=(not with_bias and c == DCH - 1),
                            )
                        if with_bias:
                            nc.tensor.matmul(
                                ps[:kh, :],
                                ones_r[0:1, :kh],
                                b_sb[0:1, :],
                                start=False, stop=True,
                            )
                        nc.vector.tensor_copy(dst[:kh, kt, :], ps[:kh, :])

                wk_sb = load_w(wk_d, HHD)
                wv_sb = load_w(wv_d, HHD)
                xkv = load_x(xkvT)
                proj_T(kT, wk_sb, bk_sb, xkv)
                proj_v(v_sb, wv_sb, bv_sb, xkv)
                wq_sb = load_w(wq_d, HHD)
                xq = load_x(xqT)
                proj_T(qT, wq_sb, bq_sb, xq)

            wo_sb = wpool.tile([128, HB, D], mt, tag="w", bufs=2)
            nc.sync.dma_start(
                wo_sb[:], mcast(wo_d.rearrange("(c p) n -> p c n", p=128))
            )

            # ---------------- phase 2+3: attention + out-proj ----------------
            st_ps = ctx.enter_context(tc.tile_pool(name="stps", bufs=2, space="PSUM"))
            pair_ps = ctx.enter_context(tc.tile_pool(name="pairps", bufs=2, space="PSUM"))
            den_ps = ctx.enter_context(tc.tile_pool(name="denps", bufs=1, space="PSUM"))
            rbo_ps = ctx.enter_context(tc.tile_pool(name="rbops", bufs=1, space="PSUM"))
            p_pool = ctx.enter_context(tc.tile_pool(name="p", bufs=8))
            an_pool = ctx.enter_context(tc.tile_pool(name="an", bufs=8))
            small = ctx.enter_context(tc.tile_pool(name="small", bufs=4))

            for (qo, cw) in QC:
                anorms = []
                for jp in range(2):  # two groups of two head-pairs
                    den = den_ps.tile([128, 512], F32, tag="den", bufs=1)
                    pairs = [
                        pair_ps.tile([128, 512], F32, tag="pair", bufs=2,
                                     name=f"pair_{jp}_{g2}")
                        for g2 in range(2)
                    ]
                    # software-pipelined: S^T/exp for step kt are emitted one
                    # step ahead of the attn@V/den consumers, so the in-order
                    # PE never waits on ScalarE's exp.
                    pend = [None, None]
                    for kt in range(NKT + 1):
                        if kt < NKT:
                            ko, kh = KT[kt]
                            for g in range(2):
                                j = jp * 2 + g
                                st = st_ps.tile([128, 2, 512], F32, tag="st",
                                                bufs=2, name=f"st_{g}")
                                nc.tensor.matmul(
                                    st[:kh, 0, :cw],
                                    kT[j][0:64, ko:ko + kh],
                                    qT[j][0:64, qo:qo + cw],
                                    start=True, stop=True,
                                )
                                nc.tensor.matmul(
                                    st[:kh, 1, :cw],
                                    kT[j][64:128, ko:ko + kh],
                                    qT[j][64:128, qo:qo + cw],
                                    start=True, stop=True,
                                )
                                p = p_pool.tile([128, 2, 512], BF16, tag="p",
                                                bufs=8, name=f"p_{g}")
                                nc.scalar.activation(
                                    p[:kh, :, :cw], st[:kh, :, :cw], AF.Exp,
                                    scale=0.125,
                                )
                                pend[g] = p
                        if kt > 0:
                            kc = kt - 1
                            ko, kh = KT[kc]
                            for g in range(2):
                                j = jp * 2 + g
                                pr = pairs[g]
                                p = pend2[g]
                                # attn @ V (bf16, col-packed in one psum tile)
                                nc.tensor.matmul(
                                    pr[0:64, :cw],
                                    v_sb[0:kh, kc, (2 * j) * 64:(2 * j) * 64 + 64],
                                    p[0:kh, 0, :cw],
                                    start=(kc == 0), stop=(kc == NKT - 1),
                                    skip_group_check=True,
                                )
                                nc.tensor.matmul(
                                    pr[64:128, :cw],
                                    v_sb[0:kh, kc, (2 * j + 1) * 64:(2 * j + 1) * 64 + 64],
                                    p[0:kh, 1, :cw],
                                    start=(kc == 0), stop=(kc == NKT - 1),
                                    skip_group_check=True,
                                )
                            for g in range(2):
                                # denominators: four M=1 matmuls col-packed
                                # into one bank (rows 0,32 pair 0; 64,96 pair 1)
                                p = pend2[g]
                                nc.tensor.matmul(
                                    den[64 * g:64 * g + 1, :cw],
                                    dones[0:kh, 0:1],
                                    p[0:kh, 0, :cw],
                                    start=(kc == 0), stop=(kc == NKT - 1),
                                    tile_position=(0, 64 * g), skip_group_check=True,
                                )
                                nc.tensor.matmul(
                                    den[64 * g + 32:64 * g + 33, :cw],
                                    dones[0:kh, 0:1],
                                    p[0:kh, 1, :cw],
                                    start=(kc == 0), stop=(kc == NKT - 1),
                                    tile_position=(0, 64 * g + 32), skip_group_check=True,
                                )
                        pend2 = list(pend)

                    # normalize both pairs: copy den rows beside their sel
                    # rows, selector-matmul broadcast, approx reciprocal,
                    # then fold into the psum->sbuf copy
                    for g in range(2):
                        nc.vector.tensor_copy(
                            ds[64 * g:64 * g + 1, :cw], den[64 * g:64 * g + 1, :cw]
                        )
                        nc.vector.tensor_copy(
                            ds[64 * g + 32:64 * g + 33, :cw],
                            den[64 * g + 32:64 * g + 33, :cw],
                        )
                    for g in range(2):
                        rb_ps = rbo_ps.tile([128, 512], F32, tag="rbo", bufs=1)
                        nc.tensor.matmul(
                            rb_ps[:, :cw],
                            sel_sb[:, g * 128:(g + 1) * 128],
                            ds[:, :cw],
                            start=True, stop=True,
                        )
                        rb_sb = small.tile([128, 512], F32, tag="rb", bufs=2)
                        nc.vector.reciprocal_approx_fast(rb_sb[:, :cw], rb_ps[:, :cw])
                        an = an_pool.tile([128, 512], mt, tag="an", bufs=8)
                        nc.vector.tensor_mul(
                            an[:, :cw], pairs[g][:, :cw], rb_sb[:, :cw]
                        )
                        anorms.append(an)

                # out-projection for this q chunk (natural [q, d] layout)
                nsub = (cw + 127) // 128
                for s in range(nsub):
                    sw = min(128, cw - s * 128)
                    for dc in range(2):
                        alt = (s * 2 + dc) % 2
                        op = (rbo_ps if alt == 0 else den_ps).tile(
                            [128, 512], F32, tag=("rbo" if alt == 0 else "den"),
                            bufs=1, name=f"op{alt}")
                        for j in range(NPAIR):
                            nc.tensor.matmul(
                                op[:sw, :],
                                anorms[j][:, s * 128:s * 128 + sw],
                                wo_sb[:, j, dc * 512:(dc + 1) * 512],
                                start=(j == 0), stop=(j == NPAIR - 1),
                            )
                        osb = small.tile([128, 512], F32, tag="os", bufs=3)
                        nc.vector.tensor_copy(osb[:sw, :], op[:sw, :])
                        nc.sync.dma_start(
                            out_d[qo + s * 128:qo + s * 128 + sw,
                                  dc * 512:(dc + 1) * 512],
                            osb[:sw, :],
                        )

    nc.compile()
    return nc


_NC = {}


def _get_nc(mode=MODE, with_bias=False):
    key = (mode, with_bias)
    if key not in _NC:
        _NC[key] = _build(mode, with_bias)
    return _NC[key]


def _sel_const():
    # sel[r, g*128 + m] routes den row r to output partitions m for pair
    # g of the group: pair 0 uses den rows 0 (->parts 0..63) and 32
    # (->64..127); pair 1 uses rows 64 and 96.
    sel = np.zeros((128, 256), np.float32)
    sel[0, 0:64] = 1.0
    sel[32, 64:128] = 1.0
    sel[64, 128:192] = 1.0
    sel[96, 192:256] = 1.0
    return sel


def _shard_inputs(mode, inputs_q, inputs_kv, Wq, bq, Wk, bk, Wv, bv, Wo, bo):
    ndt = ml_dtypes.bfloat16 if mode == "bf16" else np.float32
    sel = _sel_const()
    ones1 = np.ones((1, 512), np.float32)
    zr = np.zeros((128, 512), np.float32)
    in_maps = []
    for b in range(B):
        xqT = np.ascontiguousarray(inputs_q[b].T).astype(ndt)
        xkvT = np.ascontiguousarray(inputs_kv[b].T).astype(ndt)
        for g in range(2):
            hs = slice(g * HG, (g + 1) * HG)
            in_maps.append({
                "xqT": xqT,
                "xkvT": xkvT,
                "wq": np.ascontiguousarray(Wq[:, hs, :].reshape(D, HHD)).astype(ndt),
                "wk": np.ascontiguousarray(Wk[:, hs, :].reshape(D, HHD)).astype(ndt),
                "wv": np.ascontiguousarray(Wv[:, hs, :].reshape(D, HHD)).astype(ndt),
                "wo": np.ascontiguousarray(Wo[hs].reshape(HHD, D)).astype(ndt),
                "bq": np.ascontiguousarray(bq[hs].reshape(1, HHD)).astype(ndt),
                "bk": np.ascontiguousarray(bk[hs].reshape(1, HHD)).astype(ndt),
                "bv": np.ascontiguousarray(bv[hs].reshape(1, HHD)).astype(ndt),
                "sel": sel,
                "ones1": ones1,
                "zr": zr,
            })
    return in_maps


def _run(inputs, trace=False, trace_kwargs=None, mode=MODE):
    inputs = {k: np.asarray(v) for k, v in inputs.items()}
    with_bias = bool(
        np.any(inputs["bq"]) or np.any(inputs["bk"]) or np.any(inputs["bv"])
    )
    nc = _get_nc(mode, with_bias)
    in_maps = _shard_inputs(mode, **inputs)
    res = run_bass_kernel_spmd(
        nc, in_maps, core_ids=list(range(2 * B)), trace=trace,
        **(trace_kwargs or {}),
    )
    bo = np.asarray(inputs["bo"], np.float32)
    out = np.empty((B, SEQ, D), np.float32)
    for b in range(B):
        out[b] = res.results[2 * b]["out"] + res.results[2 * b + 1]["out"] + bo
    return out, res


def kernel(**inputs):
    out, _ = _run(inputs, trace=False)
    return out


# revision 17
# speedup vs baseline: 1.1377x; 1.1377x over previous
"""Trainium2 Bass kernel for flax MultiHeadDotProductAttention.

Shapes (hardcoded): B=4, Q=K=1500, D=1024, H=16, HD=64.
Sharding: 8 cores = 4 batches x 2 head-groups (8 heads each).
Each core computes its batch's attention output for its 8 heads plus the
output projection restricted to those heads; the host sums the two
head-group partials per batch and adds bo.

Dataflow per core (all layouts chosen so no on-device transposes are
needed; host passes x pre-transposed):
  qT/kT [hhd, seq] and v [seq, hhd] via projection matmuls;
  S^T[k,q] = kT.T-slices @ qT (K=64, row-packed 2 heads per PE slot);
  P^T = exp(S^T/8) on ScalarE (psum->sbuf, bf16);
  attn_outT += v_tile.T @ P^T (bf16, col-packed 2 heads per slot) and
  denominators via ones-vector matmuls (4 heads col-packed per slot),
  two head-pairs interleaved per k step so PE has independent work
  while ScalarE exponentiates; normalization via a selector matmul
  broadcast + one full-width approximate reciprocal; out-projection
  consumes the normalized [hhd, q] tiles as stationary operands ->
  natural [q, d] output tiles DMA'd straight to HBM.

MODE: "bf16" (default) runs all big matmuls in bf16 (weight loads
overlap in-flight matmuls); "mixed" keeps projections/S^T/out-proj in
fp32r (higher precision, but each matmul pays a serialized weight load).
"""

import os
import sys

sys.path.insert(0, "/opt/trn_rl_repo")

import numpy as np  # noqa: E402
import ml_dtypes  # noqa: E402
import concourse.bacc as bacc  # noqa: E402
import concourse.mybir as mybir  # noqa: E402
import concourse.tile as tile  # noqa: E402
from concourse.bass_utils import run_bass_kernel_spmd  # noqa: E402

F32 = mybir.dt.float32
F32R = mybir.dt.float32r
BF16 = mybir.dt.bfloat16
AF = mybir.ActivationFunctionType

B, SEQ, D, H, HD = 4, 1500, 1024, 16, 64
HG = 8                      # heads per group
HHD = HG * HD               # 512
DCH = D // 128              # 8 d-chunks
HB = HHD // 128             # 4 hhd blocks (2 heads each)
NPAIR = HB                  # 4 head pairs per group
QC = [(0, 512), (512, 512), (1024, 476)]          # q chunks
KT = [(i * 128, min(128, SEQ - i * 128)) for i in range((SEQ + 127) // 128)]
NKT = len(KT)               # 12 (last tile 92 rows)

MODE = os.environ.get("BASS_MM_DTYPE", "bf16")


def _build(mode, with_bias):
    mt = BF16 if mode == "bf16" else F32R          # big-matmul operand dtype
    MTD = BF16 if mode == "bf16" else F32          # dram dtype for x/w/b

    nc = bacc.Bacc("TRN2", target_bir_lowering=False, debug=False, num_devices=8)

    xqT = nc.declare_dram_parameter("xqT", [D, SEQ], MTD, isOutput=False)
    xkvT = nc.declare_dram_parameter("xkvT", [D, SEQ], MTD, isOutput=False)
    wq_d = nc.declare_dram_parameter("wq", [D, HHD], MTD, isOutput=False)
    wk_d = nc.declare_dram_parameter("wk", [D, HHD], MTD, isOutput=False)
    wv_d = nc.declare_dram_parameter("wv", [D, HHD], MTD, isOutput=False)
    wo_d = nc.declare_dram_parameter("wo", [HHD, D], MTD, isOutput=False)
    bq_d = nc.declare_dram_parameter("bq", [1, HHD], MTD, isOutput=False)
    bk_d = nc.declare_dram_parameter("bk", [1, HHD], MTD, isOutput=False)
    bv_d = nc.declare_dram_parameter("bv", [1, HHD], MTD, isOutput=False)
    sel_d = nc.declare_dram_parameter("sel", [128, 256], F32, isOutput=False)
    ones_d = nc.declare_dram_parameter("ones1", [1, 512], F32, isOutput=False)
    zr_d = nc.declare_dram_parameter("zr", [128, 512], F32, isOutput=False)
    out_d = nc.declare_dram_parameter("out", [SEQ, D], F32, isOutput=True)

    def mcast(ap):
        # view a dram param as the matmul dtype
        return ap if mode == "bf16" else ap.bitcast(F32R)

    with tile.TileContext(nc) as tc:
        from contextlib import ExitStack

        with ExitStack() as ctx:
            ctx.enter_context(nc.allow_low_precision(
                reason="bf16/f32r matmul operands; psum accumulation is fp32"
            ))
            const = ctx.enter_context(tc.tile_pool(name="const", bufs=1))
            ones_r = const.tile([1, 512], mt, tag="ones")
            if mode == "bf16":
                nc.vector.memset(ones_r[:], 1.0)
            else:
                nc.sync.dma_start(ones_r[:], ones_d[:].bitcast(F32R))
            dones = const.tile([128, 1], BF16, tag="dones")
            nc.vector.memset(dones[:], 1.0)
            sel_sb = const.tile([128, 256], F32R, tag="sel")
            nc.sync.dma_start(sel_sb[:], sel_d[:].bitcast(F32R))
            ds = const.tile([128, 512], F32R, tag="ds")
            nc.sync.dma_start(ds[:], zr_d[:].bitcast(F32R))
            bq_sb = const.tile([1, HHD], mt, tag="bq")
            nc.sync.dma_start(bq_sb[:], mcast(bq_d[:]))
            bk_sb = const.tile([1, HHD], mt, tag="bk")
            nc.sync.dma_start(bk_sb[:], mcast(bk_d[:]))
            bv_sb = const.tile([1, HHD], mt, tag="bv")
            nc.sync.dma_start(bv_sb[:], mcast(bv_d[:]))

            # persistent activations for the attention phase
            qT = [const.tile([128, SEQ], mt, tag="qT", name=f"qT{i}", bufs=HB)
                  for i in range(HB)]                       # [hhd%128, q] per blk
            kT = [const.tile([128, SEQ], mt, tag="kT", name=f"kT{i}", bufs=HB)
                  for i in range(HB)]
            v_sb = const.tile([128, NKT, HHD], BF16, tag="v")  # [k%128, ktile, hhd]

            wpool = ctx.enter_context(tc.tile_pool(name="w", bufs=2))

            def load_w(dram, cols):
                t = wpool.tile([128, D // 128, cols], mt, tag="w", bufs=2)
                nc.sync.dma_start(
                    t[:], mcast(dram.rearrange("(c p) n -> p c n", p=128))
                )
                return t

            # ---------------- phase 1: projections ----------------
            with tc.tile_pool(name="x", bufs=8) as xpool, \
                 tc.tile_pool(name="mmps", bufs=2, space="PSUM") as mm_ps:

                def load_x(dram):
                    xs = []
                    for c in range(DCH):
                        t = xpool.tile([128, SEQ], mt, tag="xc", bufs=8)
                        nc.sync.dma_start(
                            t[:], mcast(dram[c * 128:(c + 1) * 128, :])
                        )
                        xs.append(t)
                    return xs

                def proj_T(dst, w_sb, b_sb, xs):
                    # dst[:, hb, q] = (x @ W + b)^T rows hb*128..+128
                    for hb in range(HB):
                        for (qo, cw) in QC:
                            ps = mm_ps.tile([128, 512], F32, tag="mm", bufs=2)
                            for c in range(DCH):
                                nc.tensor.matmul(
                                    ps[:, :cw],
                                    w_sb[:, c, hb * 128:(hb + 1) * 128],
                                    xs[c][:, qo:qo + cw],
                                    start=(c == 0), stop=(not with_bias and c == DCH - 1),
                                )
                            if with_bias:
                                nc.tensor.matmul(
                                    ps[:, :cw],
                                    b_sb[0:1, hb * 128:(hb + 1) * 128],
                                    ones_r[0:1, :cw],
                                    start=False, stop=True,
                                )
                            nc.vector.tensor_copy(
                                dst[hb][:, qo:qo + cw], ps[:, :cw]
                            )

                def proj_v(dst, w_sb, b_sb, xs):
                    # dst[:, kt, hhd] = (x @ W + b) rows kt*128..
                    for kt, (ko, kh) in enumerate(KT):
                        ps = mm_ps.tile([128, 512], F32, tag="mm", bufs=2)
                        for c in range(DCH):
                            nc.tensor.matmul(
                                ps[:kh, :],
                                xs[c][:, ko:ko + kh],
                                w_sb[:, c, :],
                                start=(c == 0), stop=(not with_bias and c == DCH - 1),
                            )
                        if with_bias:
                            nc.tensor.matmul(
                                ps[:kh, :],
                                ones_r[0:1, :kh],
                                b_sb[0:1, :],
                                start=False, stop=True,
                            )
                        nc.vector.tensor_copy(dst[:kh, kt, :], ps[:kh, :])

                wk_sb = load_w(wk_d, HHD)
                wv_sb = load_w(wv_d, HHD)
                xkv = load_x(xkvT)
                proj_T(kT, wk_sb, bk_sb, xkv)
                proj_v(v_sb, wv_sb, bv_sb, xkv)
                wq_sb = load_w(wq_d, HHD)
                xq = load_x(xqT)
                proj_T(qT, wq_sb, bq_sb, xq)

            wo_sb = wpool.tile([128, HB, D], mt, tag="w", bufs=2)
            nc.sync.dma_start(
                wo_sb[:], mcast(wo_d.rearrange("(c p) n -> p c n", p=128))
            )

            # ---------------- phase 2+3: attention + out-proj ----------------
            st_ps = ctx.enter_context(tc.tile_pool(name="stps", bufs=2, space="PSUM"))
            pair_ps = ctx.enter_context(tc.tile_pool(name="pairps", bufs=2, space="PSUM"))
            den_ps = ctx.enter_context(tc.tile_pool(name="denps", bufs=1, space="PSUM"))
            rbo_ps = ctx.enter_context(tc.tile_pool(name="rbops", bufs=1, space="PSUM"))
            p_pool = ctx.enter_context(tc.tile_pool(name="p", bufs=8))
            an_pool = ctx.enter_context(tc.tile_pool(name="an", bufs=8))
            small = ctx.enter_context(tc.tile_pool(name="small", bufs=4))

            for (qo, cw) in QC:
                anorms = []
                for jp in range(2):  # two groups of two head-pairs
                    den = den_ps.tile([128, 512], F32, tag="den", bufs=1)
                    pairs = [
                        pair_ps.tile([128, 512], F32, tag="pair", bufs=2,
                                     name=f"pair_{jp}_{g2}")
                        for g2 in range(2)
                    ]
                    # software-pipelined: S^T/exp for step kt are emitted one
                    # step ahead of the attn@V/den consumers, so the in-order
                    # PE never waits on ScalarE's exp.
                    pend = [None, None]
                    for kt in range(NKT + 1):
                        if kt < NKT:
                            ko, kh = KT[kt]
                            for g in range(2):
                                j = jp * 2 + g
                                st = st_ps.tile([128, 2, 512], F32, tag="st",
                                                bufs=2, name=f"st_{g}")
                                nc.tensor.matmul(
                                    st[:kh, 0, :cw],
                                    kT[j][0:64, ko:ko + kh],
                                    qT[j][0:64, qo:qo + cw],
                                    start=True, stop=True,
                                )
                                nc.tensor.matmul(
                                    st[:kh, 1, :cw],
                                    kT[j][64:128, ko:ko + kh],
                                    qT[j][64:128, qo:qo + cw],
                                    start=True, stop=True,
                                )
                                p = p_pool.tile([128, 2, 512], BF16, tag="p",
                                                bufs=8, name=f"p_{g}")
                                nc.scalar.activation(
                                    p[:kh, :, :cw], st[:kh, :, :cw], AF.Exp,
                                    scale=0.125,
                                )
                                pend[g] = p
                        if kt > 0:
                            kc = kt - 1
                            ko, kh = KT[kc]
                            for g in range(2):
                                j = jp * 2 + g
                                pr = pairs[g]
                                p = pend2[g]
                                # attn @ V (bf16, col-packed in one psum tile)
                                nc.tensor.matmul(
                                    pr[0:64, :cw],
                                    v_sb[0:kh, kc, (2 * j) * 64:(2 * j) * 64 + 64],
                                    p[0:kh, 0, :cw],
                                    start=(kc == 0), stop=(kc == NKT - 1),
                                    skip_group_check=True,
                                )
                                nc.tensor.matmul(
                                    pr[64:128, :cw],
                                    v_sb[0:kh, kc, (2 * j + 1) * 64:(2 * j + 1) * 64 + 64],
                                    p[0:kh, 1, :cw],
                                    start=(kc == 0), stop=(kc == NKT - 1),
                                    skip_group_check=True,
                                )
                            for g in range(2):
                                # denominators: four M=1 matmuls col-packed
                                # into one bank (rows 0,32 pair 0; 64,96 pair 1)
                                p = pend2[g]
                                nc.tensor.matmul(
                                    den[64 * g:64 * g + 1, :cw],
                                    dones[0:kh, 0:1],
                                    p[0:kh, 0, :cw],
                                    start=(kc == 0), stop=(kc == NKT - 1),
                                    tile_position=(0, 64 * g), skip_group_check=True,
                                )
                                nc.tensor.matmul(
                                    den[64 * g + 32:64 * g + 33, :cw],
                                    dones[0:kh, 0:1],
                                    p[0:kh, 1, :cw],
                                    start=(kc == 0), stop=(kc == NKT - 1),
                                    tile_position=(0, 64 * g + 32), skip_group_check=True,
                                )
                        pend2 = list(pend)

                    # normalize both pairs: copy den rows beside their sel
                    # rows, selector-matmul broadcast, approx reciprocal,
                    # then fold into the psum->sbuf copy
                    for g in range(2):
                        nc.vector.tensor_copy(
                            ds[64 * g:64 * g + 1, :cw], den[64 * g:64 * g + 1, :cw]
                        )
                        nc.vector.tensor_copy(
                            ds[64 * g + 32:64 * g + 33, :cw],
                            den[64 * g + 32:64 * g + 33, :cw],
                        )
                    for g in range(2):
                        rb_ps = rbo_ps.tile([128, 512], F32, tag="rbo", bufs=1)
                        nc.tensor.matmul(
                            rb_ps[:, :cw],
                            sel_sb[:, g * 128:(g + 1) * 128],
                            ds[:, :cw],
                            start=True, stop=True,
                        )
                        rb_sb = small.tile([128, 512], F32, tag="rb", bufs=2)
                        nc.vector.reciprocal_approx_fast(rb_sb[:, :cw], rb_ps[:, :cw])
                        an = an_pool.tile([128, 512], mt, tag="an", bufs=8)
                        nc.vector.tensor_mul(
                            an[:, :cw], pairs[g][:, :cw], rb_sb[:, :cw]
                        )
                        anorms.append(an)

                # out-projection for this q chunk (natural [q, d] layout)
                nsub = (cw + 127) // 128
                for s in range(nsub):
                    sw = min(128, cw - s * 128)
                    for dc in range(2):
                        op = rbo_ps.tile([128, 512], F32, tag="rbo", bufs=1)
                        for j in range(NPAIR):
                            nc.tensor.matmul(
                                op[:sw, :],
                                anorms[j][:, s * 128:s * 128 + sw],
                                wo_sb[:, j, dc * 512:(dc + 1) * 512],
                                start=(j == 0), stop=(j == NPAIR - 1),
                            )
                        osb = small.tile([128, 512], F32, tag="os", bufs=3)
                        nc.vector.tensor_copy(osb[:sw, :], op[:sw, :])
                        nc.sync.dma_start(
                            out_d[qo + s * 128:qo + s * 128 + sw,
                                  dc * 512:(dc + 1) * 512],
                            osb[:sw, :],
                        )

    nc.compile()
    return nc


_NC = {}


def _get_nc(mode=MODE, with_bias=False):
    key = (mode, with_bias)
    if key not in _NC:
        _NC[key] = _build(mode, with_bias)
    return _NC[key]


def _sel_const():
    # sel[r, g*128 + m] routes den row r to output partitions m for pair
    # g of the group: pair 0 uses den rows 0 (->parts 0..63) and 32
    # (->64..127); pair 1 uses rows 64 and 96.
    sel = np.zeros((128, 256), np.float32)
    sel[0, 0:64] = 1.0
    sel[32, 64:128] = 1.0
    sel[64, 128:192] = 1.0
    sel[96, 192:256] = 1.0
    return sel


def _shard_inputs(mode, inputs_q, inputs_kv, Wq, bq, Wk, bk, Wv, bv, Wo, bo):
    ndt = ml_dtypes.bfloat16 if mode == "bf16" else np.float32
    sel = _sel_const()
    ones1 = np.ones((1, 512), np.float32)
    zr = np.zeros((128, 512), np.float32)
    in_maps = []
    for b in range(B):
        xqT = np.ascontiguousarray(inputs_q[b].T).astype(ndt)
        xkvT = np.ascontiguousarray(inputs_kv[b].T).astype(ndt)
        for g in range(2):
            hs = slice(g * HG, (g + 1) * HG)
            in_maps.append({
                "xqT": xqT,
                "xkvT": xkvT,
                "wq": np.ascontiguousarray(Wq[:, hs, :].reshape(D, HHD)).astype(ndt),
                "wk": np.ascontiguousarray(Wk[:, hs, :].reshape(D, HHD)).astype(ndt),
                "wv": np.ascontiguousarray(Wv[:, hs, :].reshape(D, HHD)).astype(ndt),
                "wo": np.ascontiguousarray(Wo[hs].reshape(HHD, D)).astype(ndt),
                "bq": np.ascontiguousarray(bq[hs].reshape(1, HHD)).astype(ndt),
                "bk": np.ascontiguousarray(bk[hs].reshape(1, HHD)).astype(ndt),
                "bv": np.ascontiguousarray(bv[hs].reshape(1, HHD)).astype(ndt),
                "sel": sel,
                "ones1": ones1,
                "zr": zr,
            })
    return in_maps


def _run(inputs, trace=False, trace_kwargs=None, mode=MODE):
    inputs = {k: np.asarray(v) for k, v in inputs.items()}
    with_bias = bool(
        np.any(inputs["bq"]) or np.any(inputs["bk"]) or np.any(inputs["bv"])
    )
    nc = _get_nc(mode, with_bias)
    in_maps = _shard_inputs(mode, **inputs)
    res = run_bass_kernel_spmd(
        nc, in_maps, core_ids=list(range(2 * B)), trace=trace,
        **(trace_kwargs or {}),
    )
    bo = np.asarray(inputs["bo"], np.float32)
    out = np.empty((B, SEQ, D), np.float32)
    for b in range(B):
        out[b] = res.results[2 * b]["out"] + res.results[2 * b + 1]["out"] + bo
    return out, res


def kernel(**inputs):
    out, _ = _run(inputs, trace=False)
    return out


# revision 19
# speedup vs baseline: 1.1618x; 1.0211x over previous
"""Trainium2 Bass kernel for flax MultiHeadDotProductAttention.

Shapes (hardcoded): B=4, Q=K=1500, D=1024, H=16, HD=64.
Sharding: 8 cores = 4 batches x 2 head-groups (8 heads each).
Each core computes its batch's attention output for its 8 heads plus the
output projection restricted to those heads; the host sums the two
head-group partials per batch and adds bo.

Dataflow per core (all layouts chosen so no on-device transposes are
needed; host passes x pre-transposed):
  qT/kT [hhd, seq] and v [seq, hhd] via projection matmuls;
  S^T[k,q] = kT.T-slices @ qT (K=64, row-packed 2 heads per PE slot);
  P^T = exp(S^T/8) on ScalarE (psum->sbuf, bf16);
  attn_outT += v_tile.T @ P^T (bf16, col-packed 2 heads per slot) and
  denominators via ones-vector matmuls (4 heads col-packed per slot),
  two head-pairs interleaved per k step so PE has independent work
  while ScalarE exponentiates; normalization via a selector matmul
  broadcast + one full-width approximate reciprocal; out-projection
  consumes the normalized [hhd, q] tiles as stationary operands ->
  natural [q, d] output tiles DMA'd straight to HBM.

MODE: "bf16" (default) runs all big matmuls in bf16 (weight loads
overlap in-flight matmuls); "mixed" keeps projections/S^T/out-proj in
fp32r (higher precision, but each matmul pays a serialized weight load).
"""

import os
import sys

sys.path.insert(0, "/opt/trn_rl_repo")

import numpy as np  # noqa: E402
import ml_dtypes  # noqa: E402
import concourse.bacc as bacc  # noqa: E402
import concourse.mybir as mybir  # noqa: E402
import concourse.tile as tile  # noqa: E402
from concourse.bass_utils import run_bass_kernel_spmd  # noqa: E402

F32 = mybir.dt.float32
F32R = mybir.dt.float32r
BF16 = mybir.dt.bfloat16
AF = mybir.ActivationFunctionType

B, SEQ, D, H, HD = 4, 1500, 1024, 16, 64
HG = 8                      # heads per group
HHD = HG * HD               # 512
DCH = D // 128              # 8 d-chunks
HB = HHD // 128             # 4 hhd blocks (2 heads each)
NPAIR = HB                  # 4 head pairs per group
QC = [(0, 512), (512, 512), (1024, 476)]          # q chunks
KT = [(i * 128, min(128, SEQ - i * 128)) for i in range((SEQ + 127) // 128)]
NKT = len(KT)               # 12 (last tile 92 rows)

MODE = os.environ.get("BASS_MM_DTYPE", "bf16")


def _build(mode, with_bias):
    mt = BF16 if mode == "bf16" else F32R          # big-matmul operand dtype
    MTD = BF16 if mode == "bf16" else F32          # dram dtype for x/w/b

    nc = bacc.Bacc("TRN2", target_bir_lowering=False, debug=False, num_devices=8)

    xqT = nc.declare_dram_parameter("xqT", [D, SEQ], MTD, isOutput=False)
    xkvT = nc.declare_dram_parameter("xkvT", [D, SEQ], MTD, isOutput=False)
    wq_d = nc.declare_dram_parameter("wq", [D, HHD], MTD, isOutput=False)
    wk_d = nc.declare_dram_parameter("wk", [D, HHD], MTD, isOutput=False)
    wv_d = nc.declare_dram_parameter("wv", [D, HHD], MTD, isOutput=False)
    wo_d = nc.declare_dram_parameter("wo", [HHD, D], MTD, isOutput=False)
    bq_d = nc.declare_dram_parameter("bq", [1, HHD], MTD, isOutput=False)
    bk_d = nc.declare_dram_parameter("bk", [1, HHD], MTD, isOutput=False)
    bv_d = nc.declare_dram_parameter("bv", [1, HHD], MTD, isOutput=False)
    sel_d = nc.declare_dram_parameter("sel", [128, 256], F32, isOutput=False)
    ones_d = nc.declare_dram_parameter("ones1", [1, 512], F32, isOutput=False)
    zr_d = nc.declare_dram_parameter("zr", [128, 512], F32, isOutput=False)
    out_d = nc.declare_dram_parameter("out", [SEQ, D], F32, isOutput=True)

    def mcast(ap):
        # view a dram param as the matmul dtype
        return ap if mode == "bf16" else ap.bitcast(F32R)

    with tile.TileContext(nc) as tc:
        from contextlib import ExitStack

        with ExitStack() as ctx:
            ctx.enter_context(nc.allow_low_precision(
                reason="bf16/f32r matmul operands; psum accumulation is fp32"
            ))
            const = ctx.enter_context(tc.tile_pool(name="const", bufs=1))
            ones_r = const.tile([1, 512], mt, tag="ones")
            if mode == "bf16":
                nc.vector.memset(ones_r[:], 1.0)
            else:
                nc.sync.dma_start(ones_r[:], ones_d[:].bitcast(F32R))
            dones = const.tile([128, 1], BF16, tag="dones")
            nc.vector.memset(dones[:], 1.0)
            sel_sb = const.tile([128, 256], F32R, tag="sel")
            nc.sync.dma_start(sel_sb[:], sel_d[:].bitcast(F32R))
            ds = const.tile([128, 512], F32R, tag="ds")
            nc.sync.dma_start(ds[:], zr_d[:].bitcast(F32R))
            bq_sb = const.tile([1, HHD], mt, tag="bq")
            nc.sync.dma_start(bq_sb[:], mcast(bq_d[:]))
            bk_sb = const.tile([1, HHD], mt, tag="bk")
            nc.sync.dma_start(bk_sb[:], mcast(bk_d[:]))
            bv_sb = const.tile([1, HHD], mt, tag="bv")
            nc.sync.dma_start(bv_sb[:], mcast(bv_d[:]))

            # persistent activations for the attention phase
            qT = const.tile([128, HB, SEQ], mt, tag="qT")      # [hhd%128, blk, q]
            kT = const.tile([128, HB, SEQ], mt, tag="kT")
            v_sb = const.tile([128, NKT, HHD], BF16, tag="v")  # [k%128, ktile, hhd]

            wpool = ctx.enter_context(tc.tile_pool(name="w", bufs=2))

            def load_w(dram, cols):
                t = wpool.tile([128, D // 128, cols], mt, tag="w", bufs=2)
                nc.sync.dma_start(
                    t[:], mcast(dram.rearrange("(c p) n -> p c n", p=128))
                )
                return t

            # ---------------- phase 1: projections ----------------
            with tc.tile_pool(name="x", bufs=8) as xpool, \
                 tc.tile_pool(name="mmps", bufs=2, space="PSUM") as mm_ps:

                def load_x(dram):
                    xs = []
                    for c in range(DCH):
                        t = xpool.tile([128, SEQ], mt, tag="xc", bufs=8)
                        nc.sync.dma_start(
                            t[:], mcast(dram[c * 128:(c + 1) * 128, :])
                        )
                        xs.append(t)
                    return xs

                def proj_T(dst, w_sb, b_sb, xs):
                    # dst[:, hb, q] = (x @ W + b)^T rows hb*128..+128
                    for hb in range(HB):
                        for (qo, cw) in QC:
                            ps = mm_ps.tile([128, 512], F32, tag="mm", bufs=2)
                            for c in range(DCH):
                                nc.tensor.matmul(
                                    ps[:, :cw],
                                    w_sb[:, c, hb * 128:(hb + 1) * 128],
                                    xs[c][:, qo:qo + cw],
                                    start=(c == 0), stop=(not with_bias and c == DCH - 1),
                                )
                            if with_bias:
                                nc.tensor.matmul(
                                    ps[:, :cw],
                                    b_sb[0:1, hb * 128:(hb + 1) * 128],
                                    ones_r[0:1, :cw],
                                    start=False, stop=True,
                                )
                            nc.vector.tensor_copy(
                                dst[:, hb, qo:qo + cw], ps[:, :cw]
                            )

                def proj_v(dst, w_sb, b_sb, xs):
                    # dst[:, kt, hhd] = (x @ W + b) rows kt*128..
                    for kt, (ko, kh) in enumerate(KT):
                        ps = mm_ps.tile([128, 512], F32, tag="mm", bufs=2)
                        for c in range(DCH):
                            nc.tensor.matmul(
                                ps[:kh, :],
                                xs[c][:, ko:ko + kh],
                                w_sb[:, c, :],
                                start=(c == 0), stop=(not with_bias and c == DCH - 1),
                            )
                        if with_bias:
                            nc.tensor.matmul(
                                ps[:kh, :],
                                ones_r[0:1, :kh],
                                b_sb[0:1, :],
                                start=False, stop=True,
                            )
                        nc.vector.tensor_copy(dst[:kh, kt, :], ps[:kh, :])

                wk_sb = load_w(wk_d, HHD)
                wv_sb = load_w(wv_d, HHD)
                xkv = load_x(xkvT)
                proj_T(kT, wk_sb, bk_sb, xkv)
                proj_v(v_sb, wv_sb, bv_sb, xkv)
                wq_sb = load_w(wq_d, HHD)
                xq = load_x(xqT)
                proj_T(qT, wq_sb, bq_sb, xq)

            wo_sb = wpool.tile([128, HB, D], mt, tag="w", bufs=2)
            nc.sync.dma_start(
                wo_sb[:], mcast(wo_d.rearrange("(c p) n -> p c n", p=128))
            )

            # ---------------- phase 2+3: attention + out-proj ----------------
            st_ps = ctx.enter_context(tc.tile_pool(name="stps", bufs=2, space="PSUM"))
            pair_ps = ctx.enter_context(tc.tile_pool(name="pairps", bufs=2, space="PSUM"))
            den_ps = ctx.enter_context(tc.tile_pool(name="denps", bufs=1, space="PSUM"))
            rbo_ps = ctx.enter_context(tc.tile_pool(name="rbops", bufs=1, space="PSUM"))
            p_pool = ctx.enter_context(tc.tile_pool(name="p", bufs=8))
            an_pool = ctx.enter_context(tc.tile_pool(name="an", bufs=8))
            small = ctx.enter_context(tc.tile_pool(name="small", bufs=4))

            for (qo, cw) in QC:
                anorms = []
                for jp in range(2):  # two groups of two head-pairs
                    den = den_ps.tile([128, 512], F32, tag="den", bufs=1)
                    pairs = [
                        pair_ps.tile([128, 512], F32, tag="pair", bufs=2,
                                     name=f"pair_{jp}_{g2}")
                        for g2 in range(2)
                    ]
                    # software-pipelined: S^T/exp for step kt are emitted one
                    # step ahead of the attn@V/den consumers, so the in-order
                    # PE never waits on ScalarE's exp.
                    pend = [None, None]
                    for kt in range(NKT + 1):
                        if kt < NKT:
                            ko, kh = KT[kt]
                            for g in range(2):
                                j = jp * 2 + g
                                st = st_ps.tile([128, 2, 512], F32, tag="st",
                                                bufs=2, name=f"st_{g}")
                                nc.tensor.matmul(
                                    st[:kh, 0, :cw],
                                    kT[0:64, j, ko:ko + kh],
                                    qT[0:64, j, qo:qo + cw],
                                    start=True, stop=True,
                                )
                                nc.tensor.matmul(
                                    st[:kh, 1, :cw],
                                    kT[64:128, j, ko:ko + kh],
                                    qT[64:128, j, qo:qo + cw],
                                    start=True, stop=True,
                                )
                                p = p_pool.tile([128, 2, 512], BF16, tag="p",
                                                bufs=8, name=f"p_{g}")
                                nc.scalar.activation(
                                    p[:kh, :, :cw], st[:kh, :, :cw], AF.Exp,
                                    scale=0.125,
                                )
                                pend[g] = p
                        if kt > 0:
                            kc = kt - 1
                            ko, kh = KT[kc]
                            for g in range(2):
                                j = jp * 2 + g
                                pr = pairs[g]
                                p = pend2[g]
                                # attn @ V (bf16, col-packed in one psum tile)
                                nc.tensor.matmul(
                                    pr[0:64, :cw],
                                    v_sb[0:kh, kc, (2 * j) * 64:(2 * j) * 64 + 64],
                                    p[0:kh, 0, :cw],
                                    start=(kc == 0), stop=(kc == NKT - 1),
                                    skip_group_check=True,
                                )
                                nc.tensor.matmul(
                                    pr[64:128, :cw],
                                    v_sb[0:kh, kc, (2 * j + 1) * 64:(2 * j + 1) * 64 + 64],
                                    p[0:kh, 1, :cw],
                                    start=(kc == 0), stop=(kc == NKT - 1),
                                    skip_group_check=True,
                                )
                            for g in range(2):
                                # denominators: four M=1 matmuls col-packed
                                # into one bank (rows 0,32 pair 0; 64,96 pair 1)
                                p = pend2[g]
                                nc.tensor.matmul(
                                    den[64 * g:64 * g + 1, :cw],
                                    dones[0:kh, 0:1],
                                    p[0:kh, 0, :cw],
                                    start=(kc == 0), stop=(kc == NKT - 1),
                                    tile_position=(0, 64 * g), skip_group_check=True,
                                )
                                nc.tensor.matmul(
                                    den[64 * g + 32:64 * g + 33, :cw],
                                    dones[0:kh, 0:1],
                                    p[0:kh, 1, :cw],
                                    start=(kc == 0), stop=(kc == NKT - 1),
                                    tile_position=(0, 64 * g + 32), skip_group_check=True,
                                )
                        pend2 = list(pend)

                    # normalize both pairs: copy den rows beside their sel
                    # rows, selector-matmul broadcast, approx reciprocal,
                    # then fold into the psum->sbuf copy
                    for g in range(2):
                        nc.vector.tensor_copy(
                            ds[64 * g:64 * g + 1, :cw], den[64 * g:64 * g + 1, :cw]
                        )
                        nc.vector.tensor_copy(
                            ds[64 * g + 32:64 * g + 33, :cw],
                            den[64 * g + 32:64 * g + 33, :cw],
                        )
                    for g in range(2):
                        rb_ps = rbo_ps.tile([128, 512], F32, tag="rbo", bufs=1)
                        nc.tensor.matmul(
                            rb_ps[:, :cw],
                            sel_sb[:, g * 128:(g + 1) * 128],
                            ds[:, :cw],
                            start=True, stop=True,
                        )
                        rb_sb = small.tile([128, 512], F32, tag="rb", bufs=2)
                        nc.vector.reciprocal_approx_fast(rb_sb[:, :cw], rb_ps[:, :cw])
                        an = an_pool.tile([128, 512], mt, tag="an", bufs=8)
                        nc.vector.tensor_mul(
                            an[:, :cw], pairs[g][:, :cw], rb_sb[:, :cw]
                        )
                        anorms.append(an)

                # out-projection for this q chunk (natural [q, d] layout)
                nsub = (cw + 127) // 128
                for s in range(nsub):
                    sw = min(128, cw - s * 128)
                    for dc in range(2):
                        op = rbo_ps.tile([128, 512], F32, tag="rbo", bufs=1)
                        for j in range(NPAIR):
                            nc.tensor.matmul(
                                op[:sw, :],
                                anorms[j][:, s * 128:s * 128 + sw],
                                wo_sb[:, j, dc * 512:(dc + 1) * 512],
                                start=(j == 0), stop=(j == NPAIR - 1),
                            )
                        osb = small.tile([128, 512], F32, tag="os", bufs=3)
                        nc.vector.tensor_copy(osb[:sw, :], op[:sw, :])
                        nc.sync.dma_start(
                            out_d[qo + s * 128:qo + s * 128 + sw,
                                  dc * 512:(dc + 1) * 512],
                            osb[:sw, :],
                        )

    nc.compile()
    return nc


_NC = {}


def _get_nc(mode=MODE, with_bias=False):
    key = (mode, with_bias)
    if key not in _NC:
        _NC[key] = _build(mode, with_bias)
    return _NC[key]


def _sel_const():
    # sel[r, g*128 + m] routes den row r to output partitions m for pair
    # g of the group: pair 0 uses den rows 0 (->parts 0..63) and 32
    # (->64..127); pair 1 uses rows 64 and 96.
    sel = np.zeros((128, 256), np.float32)
    sel[0, 0:64] = 1.0
    sel[32, 64:128] = 1.0
    sel[64, 128:192] = 1.0
    sel[96, 192:256] = 1.0
    return sel


def _shard_inputs(mode, inputs_q, inputs_kv, Wq, bq, Wk, bk, Wv, bv, Wo, bo):
    ndt = ml_dtypes.bfloat16 if mode == "bf16" else np.float32
    sel = _sel_const()
    ones1 = np.ones((1, 512), np.float32)
    zr = np.zeros((128, 512), np.float32)
    in_maps = []
    for b in range(B):
        xqT = np.ascontiguousarray(inputs_q[b].T).astype(ndt)
        xkvT = np.ascontiguousarray(inputs_kv[b].T).astype(ndt)
        for g in range(2):
            hs = slice(g * HG, (g + 1) * HG)
            in_maps.append({
                "xqT": xqT,
                "xkvT": xkvT,
                "wq": np.ascontiguousarray(Wq[:, hs, :].reshape(D, HHD)).astype(ndt),
                "wk": np.ascontiguousarray(Wk[:, hs, :].reshape(D, HHD)).astype(ndt),
                "wv": np.ascontiguousarray(Wv[:, hs, :].reshape(D, HHD)).astype(ndt),
                "wo": np.ascontiguousarray(Wo[hs].reshape(HHD, D)).astype(ndt),
                "bq": np.ascontiguousarray(bq[hs].reshape(1, HHD)).astype(ndt),
                "bk": np.ascontiguousarray(bk[hs].reshape(1, HHD)).astype(ndt),
                "bv": np.ascontiguousarray(bv[hs].reshape(1, HHD)).astype(ndt),
                "sel": sel,
                "ones1": ones1,
                "zr": zr,
            })
    return in_maps


def _run(inputs, trace=False, trace_kwargs=None, mode=MODE):
    inputs = {k: np.asarray(v) for k, v in inputs.items()}
    with_bias = bool(
        np.any(inputs["bq"]) or np.any(inputs["bk"]) or np.any(inputs["bv"])
    )
    nc = _get_nc(mode, with_bias)
    in_maps = _shard_inputs(mode, **inputs)
    res = run_bass_kernel_spmd(
        nc, in_maps, core_ids=list(range(2 * B)), trace=trace,
        **(trace_kwargs or {}),
    )
    bo = np.asarray(inputs["bo"], np.float32)
    out = np.empty((B, SEQ, D), np.float32)
    for b in range(B):
        out[b] = res.results[2 * b]["out"] + res.results[2 * b + 1]["out"] + bo
    return out, res


def kernel(**inputs):
    out, _ = _run(inputs, trace=False)
    return out


# revision 22
# speedup vs baseline: 1.1729x; 1.0096x over previous
"""Trainium2 Bass kernel for flax MultiHeadDotProductAttention.

Shapes (hardcoded): B=4, Q=K=1500, D=1024, H=16, HD=64.
Sharding: 8 cores = 4 batches x 2 head-groups (8 heads each).
Each core computes its batch's attention output for its 8 heads plus the
output projection restricted to those heads; the host sums the two
head-group partials per batch and adds bo.

Dataflow per core (all layouts chosen so no on-device transposes are
needed; host passes x pre-transposed):
  qT/kT [hhd, seq] and v [seq, hhd] via projection matmuls;
  S^T[k,q] = kT.T-slices @ qT (K=64, row-packed 2 heads per PE slot);
  P^T = exp(S^T/8) on ScalarE (psum->sbuf, bf16);
  attn_outT += v_tile.T @ P^T (bf16, col-packed 2 heads per slot) and
  denominators via ones-vector matmuls (4 heads col-packed per slot),
  two head-pairs interleaved per k step so PE has independent work
  while ScalarE exponentiates; normalization via a selector matmul
  broadcast + one full-width approximate reciprocal; out-projection
  consumes the normalized [hhd, q] tiles as stationary operands ->
  natural [q, d] output tiles DMA'd straight to HBM.

MODE: "bf16" (default) runs all big matmuls in bf16 (weight loads
overlap in-flight matmuls); "mixed" keeps projections/S^T/out-proj in
fp32r (higher precision, but each matmul pays a serialized weight load).
"""

import os
import sys

sys.path.insert(0, "/opt/trn_rl_repo")

import numpy as np  # noqa: E402
import ml_dtypes  # noqa: E402
import concourse.bacc as bacc  # noqa: E402
import concourse.mybir as mybir  # noqa: E402
import concourse.tile as tile  # noqa: E402
from concourse.bass_utils import run_bass_kernel_spmd  # noqa: E402

F32 = mybir.dt.float32
F32R = mybir.dt.float32r
BF16 = mybir.dt.bfloat16
AF = mybir.ActivationFunctionType

B, SEQ, D, H, HD = 4, 1500, 1024, 16, 64
HG = 8                      # heads per group
HHD = HG * HD               # 512
DCH = D // 128              # 8 d-chunks
HB = HHD // 128             # 4 hhd blocks (2 heads each)
NPAIR = HB                  # 4 head pairs per group
QC = [(0, 512), (512, 512), (1024, 476)]          # q chunks
KT = [(i * 128, min(128, SEQ - i * 128)) for i in range((SEQ + 127) // 128)]
NKT = len(KT)               # 12 (last tile 92 rows)

MODE = os.environ.get("BASS_MM_DTYPE", "bf16")


def _build(mode, with_bias):
    mt = BF16 if mode == "bf16" else F32R          # big-matmul operand dtype
    MTD = BF16 if mode == "bf16" else F32          # dram dtype for x/w/b

    nc = bacc.Bacc("TRN2", target_bir_lowering=False, debug=False, num_devices=8)

    xqT = nc.declare_dram_parameter("xqT", [D, SEQ], MTD, isOutput=False)
    xkvT = nc.declare_dram_parameter("xkvT", [D, SEQ], MTD, isOutput=False)
    wq_d = nc.declare_dram_parameter("wq", [D, HHD], MTD, isOutput=False)
    wk_d = nc.declare_dram_parameter("wk", [D, HHD], MTD, isOutput=False)
    wv_d = nc.declare_dram_parameter("wv", [D, HHD], MTD, isOutput=False)
    wo_d = nc.declare_dram_parameter("wo", [HHD, D], MTD, isOutput=False)
    bq_d = nc.declare_dram_parameter("bq", [1, HHD], MTD, isOutput=False)
    bk_d = nc.declare_dram_parameter("bk", [1, HHD], MTD, isOutput=False)
    bv_d = nc.declare_dram_parameter("bv", [1, HHD], MTD, isOutput=False)
    sel_d = nc.declare_dram_parameter("sel", [128, 64], F32, isOutput=False)
    ones_d = nc.declare_dram_parameter("ones1", [1, 512], F32, isOutput=False)
    zr_d = nc.declare_dram_parameter("zr", [128, 512], F32, isOutput=False)
    out_d = nc.declare_dram_parameter("out", [SEQ, D], F32, isOutput=True)

    def mcast(ap):
        # view a dram param as the matmul dtype
        return ap if mode == "bf16" else ap.bitcast(F32R)

    with tile.TileContext(nc) as tc:
        from contextlib import ExitStack

        with ExitStack() as ctx:
            ctx.enter_context(nc.allow_low_precision(
                reason="bf16/f32r matmul operands; psum accumulation is fp32"
            ))
            const = ctx.enter_context(tc.tile_pool(name="const", bufs=1))
            ones_r = const.tile([1, 512], mt, tag="ones")
            if mode == "bf16":
                nc.vector.memset(ones_r[:], 1.0)
            else:
                nc.sync.dma_start(ones_r[:], ones_d[:].bitcast(F32R))
            sel_sb = const.tile([128, 64], F32R, tag="sel")
            nc.sync.dma_start(sel_sb[:], sel_d[:].bitcast(F32R))
            ds_e = const.tile([128, 512], F32R, tag="dse")
            nc.sync.dma_start(ds_e[:], zr_d[:].bitcast(F32R))
            ds_o = const.tile([128, 512], F32R, tag="dso")
            nc.sync.dma_start(ds_o[:], zr_d[:].bitcast(F32R))
            bq_sb = const.tile([1, HHD], mt, tag="bq")
            nc.sync.dma_start(bq_sb[:], mcast(bq_d[:]))
            bk_sb = const.tile([1, HHD], mt, tag="bk")
            nc.sync.dma_start(bk_sb[:], mcast(bk_d[:]))
            bv_sb = const.tile([1, HHD], mt, tag="bv")
            nc.sync.dma_start(bv_sb[:], mcast(bv_d[:]))

            # persistent activations for the attention phase
            qT = const.tile([128, HB, SEQ], mt, tag="qT")      # [hhd%128, blk, q]
            kT = const.tile([128, HB, SEQ], mt, tag="kT")
            v_sb = const.tile([128, NKT, HG, 65], BF16, tag="v")
            # [k%128, ktile, head, hd|1]: 65th column is ones so the attn@V
            # matmul also accumulates the softmax denominator into row 64
            nc.vector.memset(v_sb[:, :, :, 64:65], 1.0)

            wpool = ctx.enter_context(tc.tile_pool(name="w", bufs=2))

            def load_w(dram, cols):
                t = wpool.tile([128, D // 128, cols], mt, tag="w", bufs=2)
                nc.sync.dma_start(
                    t[:], mcast(dram.rearrange("(c p) n -> p c n", p=128))
                )
                return t

            # ---------------- phase 1: projections ----------------
            with tc.tile_pool(name="x", bufs=8) as xpool, \
                 tc.tile_pool(name="mmps", bufs=2, space="PSUM") as mm_ps:

                def load_x(dram):
                    xs = []
                    for c in range(DCH):
                        t = xpool.tile([128, SEQ], mt, tag="xc", bufs=8)
                        nc.sync.dma_start(
                            t[:], mcast(dram[c * 128:(c + 1) * 128, :])
                        )
                        xs.append(t)
                    return xs

                def proj_T(dst, w_sb, b_sb, xs):
                    # dst[:, hb, q] = (x @ W + b)^T rows hb*128..+128
                    for hb in range(HB):
                        for (qo, cw) in QC:
                            ps = mm_ps.tile([128, 512], F32, tag="mm", bufs=2)
                            for c in range(DCH):
                                nc.tensor.matmul(
                                    ps[:, :cw],
                                    w_sb[:, c, hb * 128:(hb + 1) * 128],
                                    xs[c][:, qo:qo + cw],
                                    start=(c == 0), stop=(not with_bias and c == DCH - 1),
                                )
                            if with_bias:
                                nc.tensor.matmul(
                                    ps[:, :cw],
                                    b_sb[0:1, hb * 128:(hb + 1) * 128],
                                    ones_r[0:1, :cw],
                                    start=False, stop=True,
                                )
                            nc.vector.tensor_copy(
                                dst[:, hb, qo:qo + cw], ps[:, :cw]
                            )

                def proj_v(dst, w_sb, b_sb, xs):
                    # dst[:, kt, hhd] = (x @ W + b) rows kt*128..
                    for kt, (ko, kh) in enumerate(KT):
                        ps = mm_ps.tile([128, 512], F32, tag="mm", bufs=2)
                        for c in range(DCH):
                            nc.tensor.matmul(
                                ps[:kh, :],
                                xs[c][:, ko:ko + kh],
                                w_sb[:, c, :],
                                start=(c == 0), stop=(not with_bias and c == DCH - 1),
                            )
                        if with_bias:
                            nc.tensor.matmul(
                                ps[:kh, :],
                                ones_r[0:1, :kh],
                                b_sb[0:1, :],
                                start=False, stop=True,
                            )
                        nc.vector.tensor_copy(
                            dst[:kh, kt, :, 0:64],
                            ps[:kh, :].rearrange("p (h c) -> p h c", c=64),
                        )

                wk_sb = load_w(wk_d, HHD)
                wv_sb = load_w(wv_d, HHD)
                xkv = load_x(xkvT)
                proj_T(kT, wk_sb, bk_sb, xkv)
                proj_v(v_sb, wv_sb, bv_sb, xkv)
                wq_sb = load_w(wq_d, HHD)
                xq = load_x(xqT)
                proj_T(qT, wq_sb, bq_sb, xq)

            wo_sb = wpool.tile([128, HB, D], mt, tag="w", bufs=2)
            nc.sync.dma_start(
                wo_sb[:], mcast(wo_d.rearrange("(c p) n -> p c n", p=128))
            )

            # ---------------- phase 2+3: attention + out-proj ----------------
            st_ps = ctx.enter_context(tc.tile_pool(name="stps", bufs=2, space="PSUM"))
            at_ps = ctx.enter_context(tc.tile_pool(name="atps", bufs=3, space="PSUM"))
            rbo_ps = ctx.enter_context(tc.tile_pool(name="rbops", bufs=1, space="PSUM"))
            p_pool = ctx.enter_context(tc.tile_pool(name="p", bufs=8))
            an_pool = ctx.enter_context(tc.tile_pool(name="an", bufs=8))
            small = ctx.enter_context(tc.tile_pool(name="small", bufs=4))

            for (qo, cw) in QC:
                anorms = []
                for j in range(NPAIR):
                    # one bank per head; rows 0..63 = attn-out^T, row 64 = den
                    pe_b = at_ps.tile([128, 512], F32, tag="attn", bufs=3,
                                      name=f"pe_{j}")
                    po_b = at_ps.tile([128, 512], F32, tag="attn", bufs=3,
                                      name=f"po_{j}")
                    # software-pipelined: S^T/exp for step kt are emitted one
                    # step ahead of the attn@V consumers, so the in-order PE
                    # never waits on ScalarE's exp.
                    pend = None
                    for kt in range(NKT + 1):
                        if kt < NKT:
                            ko, kh = KT[kt]
                            st = st_ps.tile([128, 2, 512], F32, tag="st", bufs=2)
                            nc.tensor.matmul(
                                st[:kh, 0, :cw],
                                kT[0:64, j, ko:ko + kh],
                                qT[0:64, j, qo:qo + cw],
                                start=True, stop=True,
                            )
                            nc.tensor.matmul(
                                st[:kh, 1, :cw],
                                kT[64:128, j, ko:ko + kh],
                                qT[64:128, j, qo:qo + cw],
                                start=True, stop=True,
                            )
                            p = p_pool.tile([128, 2, 512], BF16, tag="p", bufs=8)
                            nc.scalar.activation(
                                p[:kh, :, :cw], st[:kh, :, :cw], AF.Exp,
                                scale=0.125,
                            )
                            pend = p
                        if kt > 0:
                            kc = kt - 1
                            ko, kh = KT[kc]
                            nc.tensor.matmul(
                                pe_b[0:65, :cw],
                                v_sb[0:kh, kc, 2 * j, :],
                                pend2[0:kh, 0, :cw],
                                start=(kc == 0), stop=(kc == NKT - 1),
                            )
                            nc.tensor.matmul(
                                po_b[0:65, :cw],
                                v_sb[0:kh, kc, 2 * j + 1, :],
                                pend2[0:kh, 1, :cw],
                                start=(kc == 0), stop=(kc == NKT - 1),
                            )
                        pend2 = pend

                    # normalize: den row -> zeroed staging tile, selector
                    # matmul broadcasts it to 64 partitions, approx recip,
                    # multiply into the normalized attn tile; the odd head is
                    # recombined into partitions 64..127 via an SBUF DMA
                    an = an_pool.tile([128, 512], mt, tag="an", bufs=8)
                    nc.vector.tensor_copy(ds_e[64:65, :cw], pe_b[64:65, :cw])
                    rb_eps = rbo_ps.tile([128, 512], F32, tag="rbo", bufs=1,
                                         name="rb_e")
                    nc.tensor.matmul(
                        rb_eps[0:64, :cw], sel_sb[:, :], ds_e[:, :cw],
                        start=True, stop=True,
                    )
                    rb_esb = small.tile([64, 512], F32, tag="rb", bufs=2,
                                        name="rb_esb")
                    nc.vector.reciprocal_approx_fast(rb_esb[:, :cw], rb_eps[0:64, :cw])
                    nc.vector.tensor_mul(
                        an[0:64, :cw], pe_b[0:64, :cw], rb_esb[:, :cw]
                    )
                    nc.vector.tensor_copy(ds_o[64:65, :cw], po_b[64:65, :cw])
                    rb_ops = rbo_ps.tile([128, 512], F32, tag="rbo", bufs=1,
                                         name="rb_o")
                    nc.tensor.matmul(
                        rb_ops[0:64, :cw], sel_sb[:, :], ds_o[:, :cw],
                        start=True, stop=True,
                    )
                    rb_osb = small.tile([64, 512], F32, tag="rb", bufs=2,
                                        name="rb_osb")
                    nc.vector.reciprocal_approx_fast(rb_osb[:, :cw], rb_ops[0:64, :cw])
                    antmp = small.tile([64, 512], mt, tag="antmp", bufs=2)
                    nc.vector.tensor_mul(
                        antmp[:, :cw], po_b[0:64, :cw], rb_osb[:, :cw]
                    )
                    nc.sync.dma_start(an[64:128, :cw], antmp[:, :cw])
                    anorms.append(an)

                # out-projection for this q chunk (natural [q, d] layout)
                nsub = (cw + 127) // 128
                for s in range(nsub):
                    sw = min(128, cw - s * 128)
                    for dc in range(2):
                        op = rbo_ps.tile([128, 512], F32, tag="rbo", bufs=1)
                        for j in range(NPAIR):
                            nc.tensor.matmul(
                                op[:sw, :],
                                anorms[j][:, s * 128:s * 128 + sw],
                                wo_sb[:, j, dc * 512:(dc + 1) * 512],
                                start=(j == 0), stop=(j == NPAIR - 1),
                            )
                        osb = small.tile([128, 512], F32, tag="os", bufs=3)
                        nc.vector.tensor_copy(osb[:sw, :], op[:sw, :])
                        nc.sync.dma_start(
                            out_d[qo + s * 128:qo + s * 128 + sw,
                                  dc * 512:(dc + 1) * 512],
                            osb[:sw, :],
                        )

    nc.compile()
    return nc


_NC = {}


def _get_nc(mode=MODE, with_bias=False):
    key = (mode, with_bias)
    if key not in _NC:
        _NC[key] = _build(mode, with_bias)
    return _NC[key]


def _sel_const():
    # broadcast matrix: den staging row 64 -> all 64 output partitions
    sel = np.zeros((128, 64), np.float32)
    sel[64, :] = 1.0
    return sel


def _shard_inputs(mode, inputs_q, inputs_kv, Wq, bq, Wk, bk, Wv, bv, Wo, bo):
    ndt = ml_dtypes.bfloat16 if mode == "bf16" else np.float32
    sel = _sel_const()
    ones1 = np.ones((1, 512), np.float32)
    zr = np.zeros((128, 512), np.float32)
    in_maps = []
    for b in range(B):
        xqT = np.ascontiguousarray(inputs_q[b].T).astype(ndt)
        xkvT = np.ascontiguousarray(inputs_kv[b].T).astype(ndt)
        for g in range(2):
            hs = slice(g * HG, (g + 1) * HG)
            in_maps.append({
                "xqT": xqT,
                "xkvT": xkvT,
                "wq": np.ascontiguousarray(Wq[:, hs, :].reshape(D, HHD)).astype(ndt),
                "wk": np.ascontiguousarray(Wk[:, hs, :].reshape(D, HHD)).astype(ndt),
                "wv": np.ascontiguousarray(Wv[:, hs, :].reshape(D, HHD)).astype(ndt),
                "wo": np.ascontiguousarray(Wo[hs].reshape(HHD, D)).astype(ndt),
                "bq": np.ascontiguousarray(bq[hs].reshape(1, HHD)).astype(ndt),
                "bk": np.ascontiguousarray(bk[hs].reshape(1, HHD)).astype(ndt),
                "bv": np.ascontiguousarray(bv[hs].reshape(1, HHD)).astype(ndt),
                "sel": sel,
                "ones1": ones1,
                "zr": zr,
            })
    return in_maps


def _run(inputs, trace=False, trace_kwargs=None, mode=MODE):
    inputs = {k: np.asarray(v) for k, v in inputs.items()}
    with_bias = bool(
        np.any(inputs["bq"]) or np.any(inputs["bk"]) or np.any(inputs["bv"])
    )
    nc = _get_nc(mode, with_bias)
    in_maps = _shard_inputs(mode, **inputs)
    res = run_bass_kernel_spmd(
        nc, in_maps, core_ids=list(range(2 * B)), trace=trace,
        **(trace_kwargs or {}),
    )
    bo = np.asarray(inputs["bo"], np.float32)
    out = np.empty((B, SEQ, D), np.float32)
    for b in range(B):
        out[b] = res.results[2 * b]["out"] + res.results[2 * b + 1]["out"] + bo
    return out, res


def kernel(**inputs):
    out, _ = _run(inputs, trace=False)
    return out


# revision 33
# speedup vs baseline: 1.3192x; 1.1248x over previous
"""Trainium2 Bass kernel for flax MultiHeadDotProductAttention.

Shapes (hardcoded): B=4, Q=K=1500, D=1024, H=16, HD=64.
Sharding: 8 cores = 4 batches x 2 head-groups (8 heads each).
Each core computes its batch's attention output for its 8 heads plus the
output projection restricted to those heads; the host sums the two
head-group partials per batch and adds bo.

Dataflow per core (all layouts chosen so no on-device transposes are
needed; host passes x pre-transposed):
  qT/kT [hhd, seq] and v [seq, hhd] via projection matmuls;
  S^T[k,q] = kT.T-slices @ qT (K=64, row-packed 2 heads per PE slot);
  P^T = exp(S^T/8) on ScalarE (psum->sbuf, bf16);
  attn_outT += v_tile.T @ P^T (bf16, col-packed 2 heads per slot) and
  denominators via ones-vector matmuls (4 heads col-packed per slot),
  two head-pairs interleaved per k step so PE has independent work
  while ScalarE exponentiates; normalization via a selector matmul
  broadcast + one full-width approximate reciprocal; out-projection
  consumes the normalized [hhd, q] tiles as stationary operands ->
  natural [q, d] output tiles DMA'd straight to HBM.

MODE: "bf16" (default) runs all big matmuls in bf16 (weight loads
overlap in-flight matmuls); "mixed" keeps projections/S^T/out-proj in
fp32r (higher precision, but each matmul pays a serialized weight load).
"""

import os
import sys

sys.path.insert(0, "/opt/trn_rl_repo")

import numpy as np  # noqa: E402
import ml_dtypes  # noqa: E402
import concourse.bacc as bacc  # noqa: E402
import concourse.mybir as mybir  # noqa: E402
import concourse.tile as tile  # noqa: E402
from concourse.bass_utils import run_bass_kernel_spmd  # noqa: E402

F32 = mybir.dt.float32
F32R = mybir.dt.float32r
BF16 = mybir.dt.bfloat16
AF = mybir.ActivationFunctionType

B, SEQ, D, H, HD = 4, 1500, 1024, 16, 64
HG = 8                      # heads per group
HHD = HG * HD               # 512
DCH = D // 128              # 8 d-chunks
HB = HHD // 128             # 4 hhd blocks (2 heads each)
NPAIR = HB                  # 4 head pairs per group
QC = [(0, 512), (512, 512), (1024, 476)]          # q chunks
KT = [(i * 128, min(128, SEQ - i * 128)) for i in range((SEQ + 127) // 128)]
NKT = len(KT)               # 12 (last tile 92 rows)

MODE = os.environ.get("BASS_MM_DTYPE", "bf16")


def _build(mode, with_bias):
    mt = BF16 if mode == "bf16" else F32R          # big-matmul operand dtype
    MTD = BF16 if mode == "bf16" else F32          # dram dtype for x/w/b

    nc = bacc.Bacc("TRN2", target_bir_lowering=False, debug=False, num_devices=8)

    xqT = nc.declare_dram_parameter("xqT", [D, SEQ], MTD, isOutput=False)
    xkvT = nc.declare_dram_parameter("xkvT", [D, SEQ], MTD, isOutput=False)
    wq_d = nc.declare_dram_parameter("wq", [D, HHD], MTD, isOutput=False)
    wk_d = nc.declare_dram_parameter("wk", [D, HHD], MTD, isOutput=False)
    wv_d = nc.declare_dram_parameter("wv", [D, HHD], MTD, isOutput=False)
    wo_d = nc.declare_dram_parameter("wo", [HHD, D], MTD, isOutput=False)
    bq_d = nc.declare_dram_parameter("bq", [1, HHD], MTD, isOutput=False)
    bk_d = nc.declare_dram_parameter("bk", [1, HHD], MTD, isOutput=False)
    bv_d = nc.declare_dram_parameter("bv", [1, HHD], MTD, isOutput=False)
    sel_d = nc.declare_dram_parameter("sel", [128, 64], F32, isOutput=False)
    ones_d = nc.declare_dram_parameter("ones1", [1, 512], F32, isOutput=False)
    zr_d = nc.declare_dram_parameter("zr", [128, 512], F32, isOutput=False)
    out_d = nc.declare_dram_parameter("out", [SEQ, D], F32, isOutput=True)

    def mcast(ap):
        # view a dram param as the matmul dtype
        return ap if mode == "bf16" else ap.bitcast(F32R)

    with tile.TileContext(nc) as tc:
        from contextlib import ExitStack

        with ExitStack() as ctx:
            ctx.enter_context(nc.allow_low_precision(
                reason="bf16/f32r matmul operands; psum accumulation is fp32"
            ))
            const = ctx.enter_context(tc.tile_pool(name="const", bufs=1))
            ones_r = const.tile([1, 512], mt, tag="ones")
            if mode == "bf16":
                nc.vector.memset(ones_r[:], 1.0)
            else:
                nc.sync.dma_start(ones_r[:], ones_d[:].bitcast(F32R))
            sel_sb = const.tile([128, 64], F32R, tag="sel")
            ds_e = const.tile([128, 512], F32R, tag="dse")
            ds_o = const.tile([128, 512], F32R, tag="dso")
            bq_sb = const.tile([1, HHD], mt, tag="bq")
            bk_sb = const.tile([1, HHD], mt, tag="bk")
            bv_sb = const.tile([1, HHD], mt, tag="bv")

            def load_consts():
                # deferred: not needed until the first normalize (~60us in),
                # so these DMAs must not delay the phase-1 weight/input loads
                nc.sync.dma_start(sel_sb[:], sel_d[:].bitcast(F32R))
                nc.sync.dma_start(ds_e[:], zr_d[:].bitcast(F32R))
                nc.sync.dma_start(ds_o[:], zr_d[:].bitcast(F32R))
                nc.sync.dma_start(bq_sb[:], mcast(bq_d[:]))
                nc.sync.dma_start(bk_sb[:], mcast(bk_d[:]))
                nc.sync.dma_start(bv_sb[:], mcast(bv_d[:]))

            # persistent activations for the attention phase
            qT_b = []
            for i in range(HB):
                qT_b.append(const.tile([128, SEQ], mt, tag=f"qT{i}",
                                       name=f"qT{i}"))  # [hhd%128, q] per block
            kT = const.tile([128, HB, SEQ], mt, tag="kT")
            # v: one tile per k-tile ([k%128, head, hd|1]) so the attn@V
            # of step kt only depends on its own projection group; the 65th
            # column is ones so attn@V also accumulates the softmax
            # denominator into row 64
            v_t = []
            for kt in range(NKT):
                vt = const.tile([128, HG, 65], BF16, tag=f"v{kt}", name=f"v{kt}")
                nc.vector.memset(vt[:, :, 64:65], 1.0)
                v_t.append(vt)

            wpool = ctx.enter_context(tc.tile_pool(name="w", bufs=2))

            def load_w(dram, cols):
                # per-d-chunk tiles: the first projection matmul only waits
                # on its own 128-row slice of the weight, not the whole DMA
                ts = []
                d3 = dram.rearrange("(c p) n -> c p n", p=128)
                for c in range(D // 128):
                    t = wpool.tile([128, cols], mt, tag="wc", bufs=24,
                                   name=f"w{c}")
                    nc.sync.dma_start(t[:], mcast(d3[c]))
                    ts.append(t)
                return ts

            # ---------------- phase 1: projections ----------------
            xpool = ctx.enter_context(tc.tile_pool(name="x", bufs=16))
            with tc.tile_pool(name="mmps", bufs=2, space="PSUM") as mm_ps:

                def load_x(dram):
                    xs = []
                    for c in range(DCH):
                        t = xpool.tile([128, SEQ], mt, tag="xc", bufs=16)
                        nc.sync.dma_start(
                            t[:], mcast(dram[c * 128:(c + 1) * 128, :])
                        )
                        xs.append(t)
                    return xs

                wk_d3 = wk_d.rearrange("(c p) n -> c p n", p=128)
                wk_sb, xkv = [], []
                for c in range(DCH):
                    t = wpool.tile([128, HHD], mt, tag="wc", bufs=24,
                                   name=f"wk{c}")
                    nc.sync.dma_start(t[:], mcast(wk_d3[c]))
                    wk_sb.append(t)
                    tx = xpool.tile([128, SEQ], mt, tag="xc", bufs=16)
                    nc.sync.dma_start(
                        tx[:], mcast(xkvT[c * 128:(c + 1) * 128, :])
                    )
                    xkv.append(tx)
                wq_sb = load_w(wq_d, HHD)
                xq = load_x(xqT)
                load_consts()

                def projT_group(dst2d, w_sb, b_sb, xs, hb, qci,
                                psum_pool, ptag, pbufs):
                    qo2, cw2 = QC[qci]
                    ps = psum_pool.tile([128, 512], F32, tag=ptag, bufs=pbufs,
                                        name=f"tps{hb}_{qci}")
                    for c in range(DCH):
                        nc.tensor.matmul(
                            ps[:, :cw2],
                            w_sb[c][:, hb * 128:(hb + 1) * 128],
                            xs[c][:, qo2:qo2 + cw2],
                            start=(c == 0), stop=(not with_bias and c == DCH - 1),
                        )
                    if with_bias:
                        nc.tensor.matmul(
                            ps[:, :cw2],
                            b_sb[0:1, hb * 128:(hb + 1) * 128],
                            ones_r[0:1, :cw2],
                            start=False, stop=True,
                        )
                    nc.vector.tensor_copy(dst2d[:, qo2:qo2 + cw2], ps[:, :cw2])

                # kT fully, then only qT block 0: attention pair j needs
                # just qT block j, so blocks 1..3 are emitted interleaved
                # into the first chunk's pairs 0..2 below
                for hb in range(HB):
                    for qci in range(len(QC)):
                        projT_group(kT[:, hb, :], wk_sb, bk_sb, xkv, hb, qci,
                                    mm_ps, "mm", 2)
                for qci in range(len(QC)):
                    projT_group(qT_b[0], wq_sb, bq_sb, xq, 0, qci,
                                mm_ps, "mm", 2)

            # v projection groups are emitted interleaved into the first
            # attention pair's k loop (see below): group kt lands exactly one
            # step before attn@V consumes v_t[kt], so the PE finishes the v
            # projection under the already-running exp pipeline.
            wv_sb = load_w(wv_d, HHD)

            def proj_v_group(kt, psum_pool, ptag, pbufs):
                ko, kh = KT[kt]
                ps = psum_pool.tile([128, 512], F32, tag=ptag, bufs=pbufs,
                                    name=f"vps{kt}")
                for c in range(DCH):
                    nc.tensor.matmul(
                        ps[:kh, :],
                        xkv[c][:, ko:ko + kh],
                        wv_sb[c][:, :],
                        start=(c == 0), stop=(not with_bias and c == DCH - 1),
                    )
                if with_bias:
                    nc.tensor.matmul(
                        ps[:kh, :],
                        ones_r[0:1, :kh],
                        bv_sb[0:1, :],
                        start=False, stop=True,
                    )
                nc.vector.tensor_copy(
                    v_t[kt][:kh, :, 0:64],
                    ps[:kh, :].rearrange("p (h c) -> p h c", c=64),
                )

            wo_sb = wpool.tile([128, HB, D], mt, tag="w", bufs=2)
            nc.sync.dma_start(
                wo_sb[:], mcast(wo_d.rearrange("(c p) n -> p c n", p=128))
            )

            # ---------------- phase 2+3: attention + out-proj ----------------
            st_ps = ctx.enter_context(tc.tile_pool(name="stps", bufs=2, space="PSUM"))
            at_ps = ctx.enter_context(tc.tile_pool(name="atps", bufs=3, space="PSUM"))
            rbo_ps = ctx.enter_context(tc.tile_pool(name="rbops", bufs=1, space="PSUM"))
            p_pool = ctx.enter_context(tc.tile_pool(name="p", bufs=20))
            an_pool = ctx.enter_context(tc.tile_pool(name="an", bufs=8))
            small = ctx.enter_context(tc.tile_pool(name="small", bufs=4))

            for (qo, cw) in QC:
                anorms = []
                for j in range(NPAIR):
                    # one bank per head; rows 0..63 = attn-out^T, row 64 = den
                    pe_b = at_ps.tile([128, 512], F32, tag="attn", bufs=3,
                                      name=f"pe_{j}")
                    po_b = at_ps.tile([128, 512], F32, tag="attn", bufs=3,
                                      name=f"po_{j}")
                    # software-pipelined: S^T/exp for step kt are emitted one
                    # step ahead of the attn@V consumers, so the in-order PE
                    # never waits on ScalarE's exp.
                    pend = None
                    for kt in range(NKT + 1):
                        if qo == 0 and j == 0 and kt < NKT:
                            # first pair of the first chunk: emit the v
                            # projection group for this k tile (uses the spare
                            # attn psum slot while only pe_b/po_b are live)
                            proj_v_group(kt, at_ps, "attn", 3)
                        if qo == 0 and j < 3 and kt in (1, 5, 9):
                            # produce qT block j+1 for the next pair
                            projT_group(qT_b[j + 1], wq_sb, bq_sb, xq,
                                        j + 1, kt // 4, at_ps, "attn", 3)
                        if kt < NKT:
                            ko, kh = KT[kt]
                            st = st_ps.tile([128, 2, 512], F32, tag="st", bufs=2)
                            nc.tensor.matmul(
                                st[:kh, 0, :cw],
                                kT[0:64, j, ko:ko + kh],
                                qT_b[j][0:64, qo:qo + cw],
                                start=True, stop=True,
                            )
                            nc.tensor.matmul(
                                st[:kh, 1, :cw],
                                kT[64:128, j, ko:ko + kh],
                                qT_b[j][64:128, qo:qo + cw],
                                start=True, stop=True,
                            )
                            p = p_pool.tile([128, 2, 512], BF16, tag="p", bufs=20)
                            nc.scalar.activation(
                                p[:kh, :, :cw], st[:kh, :, :cw], AF.Exp,
                                scale=0.125,
                            )
                            pend = p
                        if kt > 0:
                            kc = kt - 1
                            ko, kh = KT[kc]
                            nc.tensor.matmul(
                                pe_b[0:65, :cw],
                                v_t[kc][0:kh, 2 * j, :],
                                pend2[0:kh, 0, :cw],
                                start=(kc == 0), stop=(kc == NKT - 1),
                            )
                            nc.tensor.matmul(
                                po_b[0:65, :cw],
                                v_t[kc][0:kh, 2 * j + 1, :],
                                pend2[0:kh, 1, :cw],
                                start=(kc == 0), stop=(kc == NKT - 1),
                            )
                        pend2 = pend

                    # normalize: den row -> zeroed staging tile, selector
                    # matmul broadcasts it to 64 partitions, approx recip,
                    # multiply into the normalized attn tile; the odd head is
                    # recombined into partitions 64..127 via an SBUF DMA
                    an = an_pool.tile([128, 512], mt, tag="an", bufs=8)
                    nc.vector.tensor_copy(ds_e[64:65, :cw], pe_b[64:65, :cw])
                    rb_eps = rbo_ps.tile([128, 512], F32, tag="rbo", bufs=1,
                                         name="rb_e")
                    nc.tensor.matmul(
                        rb_eps[0:64, :cw], sel_sb[:, :], ds_e[:, :cw],
                        start=True, stop=True,
                    )
                    rb_esb = small.tile([64, 512], F32, tag="rb", bufs=2,
                                        name="rb_esb")
                    nc.vector.reciprocal_approx_fast(rb_esb[:, :cw], rb_eps[0:64, :cw])
                    nc.vector.tensor_mul(
                        an[0:64, :cw], pe_b[0:64, :cw], rb_esb[:, :cw]
                    )
                    nc.vector.tensor_copy(ds_o[64:65, :cw], po_b[64:65, :cw])
                    rb_ops = rbo_ps.tile([128, 512], F32, tag="rbo", bufs=1,
                                         name="rb_o")
                    nc.tensor.matmul(
                        rb_ops[0:64, :cw], sel_sb[:, :], ds_o[:, :cw],
                        start=True, stop=True,
                    )
                    rb_osb = small.tile([64, 512], F32, tag="rb", bufs=2,
                                        name="rb_osb")
                    nc.vector.reciprocal_approx_fast(rb_osb[:, :cw], rb_ops[0:64, :cw])
                    antmp = small.tile([64, 512], mt, tag="antmp", bufs=2)
                    nc.vector.tensor_mul(
                        antmp[:, :cw], po_b[0:64, :cw], rb_osb[:, :cw]
                    )
                    nc.sync.dma_start(an[64:128, :cw], antmp[:, :cw])
                    anorms.append(an)

                # out-projection for this q chunk (natural [q, d] layout)
                last_chunk = qo == QC[-1][0]
                nsub = (cw + 127) // 128
                for s in range(nsub):
                    sw = min(128, cw - s * 128)
                    for dc in range(2):
                        alt = last_chunk and (s * 2 + dc) % 2 == 1
                        op = (st_ps if alt else rbo_ps).tile(
                            [128, 512], F32, tag=("st" if alt else "rbo"),
                            bufs=(2 if alt else 1), name=f"op{int(alt)}")
                        for j in range(NPAIR):
                            nc.tensor.matmul(
                                op[:sw, :],
                                anorms[j][:, s * 128:s * 128 + sw],
                                wo_sb[:, j, dc * 512:(dc + 1) * 512],
                                start=(j == 0), stop=(j == NPAIR - 1),
                            )
                        osb = small.tile([128, 512], F32, tag="os", bufs=3)
                        if last_chunk:
                            # ScalarE is idle after the last exp; keep DVE
                            # free for the trailing normalize ops
                            nc.scalar.copy(osb[:sw, :], op[:sw, :])
                        else:
                            nc.vector.tensor_copy(osb[:sw, :], op[:sw, :])
                        nc.sync.dma_start(
                            out_d[qo + s * 128:qo + s * 128 + sw,
                                  dc * 512:(dc + 1) * 512],
                            osb[:sw, :],
                        )

    nc.compile()
    return nc


_NC = {}


def _get_nc(mode=MODE, with_bias=False):
    key = (mode, with_bias)
    if key not in _NC:
        _NC[key] = _build(mode, with_bias)
    return _NC[key]


def _sel_const():
    # broadcast matrix: den staging row 64 -> all 64 output partitions
    sel = np.zeros((128, 64), np.float32)
    sel[64, :] = 1.0
    return sel


def _shard_inputs(mode, inputs_q, inputs_kv, Wq, bq, Wk, bk, Wv, bv, Wo, bo):
    ndt = ml_dtypes.bfloat16 if mode == "bf16" else np.float32
    sel = _sel_const()
    ones1 = np.ones((1, 512), np.float32)
    zr = np.zeros((128, 512), np.float32)
    in_maps = []
    for b in range(B):
        xqT = np.ascontiguousarray(inputs_q[b].T).astype(ndt)
        xkvT = np.ascontiguousarray(inputs_kv[b].T).astype(ndt)
        for g in range(2):
            hs = slice(g * HG, (g + 1) * HG)
            in_maps.append({
                "xqT": xqT,
                "xkvT": xkvT,
                "wq": np.ascontiguousarray(Wq[:, hs, :].reshape(D, HHD)).astype(ndt),
                "wk": np.ascontiguousarray(Wk[:, hs, :].reshape(D, HHD)).astype(ndt),
                "wv": np.ascontiguousarray(Wv[:, hs, :].reshape(D, HHD)).astype(ndt),
                "wo": np.ascontiguousarray(Wo[hs].reshape(HHD, D)).astype(ndt),
                "bq": np.ascontiguousarray(bq[hs].reshape(1, HHD)).astype(ndt),
                "bk": np.ascontiguousarray(bk[hs].reshape(1, HHD)).astype(ndt),
                "bv": np.ascontiguousarray(bv[hs].reshape(1, HHD)).astype(ndt),
                "sel": sel,
                "ones1": ones1,
                "zr": zr,
            })
    return in_maps


def _run(inputs, trace=False, trace_kwargs=None, mode=MODE):
    inputs = {k: np.asarray(v) for k, v in inputs.items()}
    with_bias = bool(
        np.any(inputs["bq"]) or np.any(inputs["bk"]) or np.any(inputs["bv"])
    )
    nc = _get_nc(mode, with_bias)
    in_maps = _shard_inputs(mode, **inputs)
    res = run_bass_kernel_spmd(
        nc, in_maps, core_ids=list(range(2 * B)), trace=trace,
        **(trace_kwargs or {}),
    )
    bo = np.asarray(inputs["bo"], np.float32)
    out = np.empty((B, SEQ, D), np.float32)
    for b in range(B):
        out[b] = res.results[2 * b]["out"] + res.results[2 * b + 1]["out"] + bo
    return out, res


def kernel(**inputs):
    out, _ = _run(inputs, trace=False)
    return out
